# revision 29
# baseline (speedup 1.0000x reference)
"""Trainium2 Bass kernel for pooled-kv sparse attention (nn_Attention).

Math (per batch element, C=512, HEADS=8, d=64, H=W=32, s=1024, n=320):
  qkv = Wqkv @ x  (1x1 conv) ; k,v pooled over space to n=320 then sigmoid
  dots = (q^T k)/8 + pos ; attn = softmax(dots) ; out = Wout @ (attn v) + bout

Device strategy (data parallel, 4 batch elements per core, 8 cores):
  - Pooling commutes with the 1x1 conv: pool x first (DVE), then project.
  - All matmuls in bf16 (PSUM accumulates f32).
  - Attention computed with n on partitions for dots (dotsT [n, s]), exp on
    ACT, multiply by host-precomputed exp(posT) on GpSimd
    (exp(dots+pos) == exp(dots)*exp(pos)), PV with s on partitions via
    lhsT=exp_attn, rhs=[v^T | 1] (ones column yields softmax denominators).
  - Normalization folded into the PSUM->SBUF copy (tensor_scalar by 1/den).
  - ao (s-partitioned) transposed to c-partitioned via DMA transpose (bf16).
"""

import os
import sys

import numpy as np
import ml_dtypes

sys.path.insert(0, "/opt/trn_rl_repo")

HEADS = 8
C = 512
HH = 32
WW = 32
S = HH * WW          # 1024
NKV = HH + WW + (HH // 2) * (WW // 2)  # 320
D = C // HEADS       # 64
B = 32
NCORES = 8
BPC = B // NCORES    # 4 batch elements per core

N_TILES = (128, 128, 64)       # n=320 split over partition tiles
N_OFFS = (0, 128, 256)


def _build_graph():
    import concourse.bass as bass
    import concourse.tile as tile
    from concourse import bacc, mybir

    f32 = mybir.dt.float32
    bf16 = mybir.dt.bfloat16
    AF = mybir.ActivationFunctionType
    ALU = mybir.AluOpType
    AX = mybir.AxisListType

    nc = bacc.Bacc("TRN2", target_bir_lowering=False, debug=False)

    x_d = nc.dram_tensor("x", [BPC, C, S], f32, kind="ExternalInput").ap()
    wq_d = nc.dram_tensor("wqT", [C, C], bf16, kind="ExternalInput").ap()
    wk_d = nc.dram_tensor("wkT", [C, C], bf16, kind="ExternalInput").ap()
    wv_d = nc.dram_tensor("wvT", [C, C], bf16, kind="ExternalInput").ap()
    wo_d = nc.dram_tensor("woutT", [C, C], bf16, kind="ExternalInput").ap()
    bo_d = nc.dram_tensor("bout", [C, 1], f32, kind="ExternalInput").ap()
    ep_d = nc.dram_tensor("eposT", [NKV, S], bf16, kind="ExternalInput").ap()
    out_d = nc.dram_tensor("out", [BPC, C, S], f32, kind="ExternalOutput").ap()

    with tile.TileContext(nc) as tc, nc.allow_low_precision("bf16 by design"):
        with (
            tc.tile_pool(name="const", bufs=1) as cpool,
            tc.tile_pool(name="xf", bufs=3) as xfpool,
            tc.tile_pool(name="xb", bufs=2) as xbpool,
            tc.tile_pool(name="xp", bufs=2) as xppool,
            tc.tile_pool(name="ptmp", bufs=3) as ptpool,
            tc.tile_pool(name="kq", bufs=2) as kqpool,
            tc.tile_pool(name="ea", bufs=3) as eapool,
            tc.tile_pool(name="eab", bufs=3) as eabpool,
            tc.tile_pool(name="ao", bufs=2) as aopool,
            tc.tile_pool(name="ofin", bufs=2) as ofpool,
            tc.tile_pool(name="vaug", bufs=2) as vpool,
            tc.tile_pool(name="ps1", bufs=2, space="PSUM") as ps1,
            tc.tile_pool(name="psd", bufs=2, space="PSUM") as psd,
            tc.tile_pool(name="psv", bufs=1, space="PSUM") as psv,
        ):
            # ---- constants (loaded once) ----
            wq_s, wk_s, wv_s, wo_s, bo_s = [], [], [], [], []
            for ct in range(4):
                for wi, (lst, dram) in enumerate(
                        ((wq_s, wq_d), (wk_s, wk_d), (wv_s, wv_d),
                         (wo_s, wo_d))):
                    t = cpool.tile([128, C], bf16, tag=f"w{wi}_{ct}")
                    nc.sync.dma_start(t[:], dram[ct * 128:(ct + 1) * 128, :])
                    lst.append(t)
                t = cpool.tile([128, 1], f32, tag=f"bo_{ct}")
                nc.sync.dma_start(t[:], bo_d[ct * 128:(ct + 1) * 128, :])
                bo_s.append(t)
            ep_s = []
            for nt in range(3):
                t = cpool.tile([N_TILES[nt], S], bf16, tag=f"ep_{nt}")
                nc.sync.dma_start(
                    t[:], ep_d[N_OFFS[nt]:N_OFFS[nt] + N_TILES[nt], :])
                ep_s.append(t)
            # per-partition sigmoid scale for v-proj n-tile 0
            vsc0 = cpool.tile([128, 1], f32, tag="vsc0")
            nc.vector.memset(vsc0[0:2 * HH, :], 1.0 / WW)
            nc.vector.memset(vsc0[2 * HH:128, :], 0.25)

            for e in range(BPC):
                # ---- load x, cast to bf16 (GpSimd) ----
                xb = []
                for ct in range(4):
                    xf = xfpool.tile([128, S], f32, tag="xf")
                    nc.gpsimd.dma_start(
                        xf[:], x_d[e, ct * 128:(ct + 1) * 128, :])
                    t = xbpool.tile([128, S], bf16, tag=f"xb{ct}")
                    nc.gpsimd.tensor_copy(t[:], xf[:])
                    xb.append(t)

                # ---- pooling -> xp [c, 320] bf16 raw SUMS (DVE) ----
                # (the 1/32, 1/32, 0.25 mean scales are folded into the
                # sigmoid activations downstream)
                xp = []
                for ct in range(4):
                    t = xppool.tile([128, NKV], bf16, tag=f"xp{ct}")
                    x3 = xb[ct][:].rearrange("p (h w) -> p h w", h=HH, w=WW)
                    # th: sum over w (innermost)
                    nc.vector.tensor_reduce(t[:, 0:HH], x3, AX.X, ALU.add)
                    # tw: sum over h (reduce innermost of permuted view)
                    xwh = xb[ct][:].rearrange("p (h w) -> p w h", h=HH, w=WW)
                    nc.vector.tensor_reduce(t[:, HH:HH + WW], xwh, AX.X,
                                            ALU.add)
                    # tp: 2x2 sum pool, two-step reduce
                    x5a = xb[ct][:].rearrange(
                        "p (haw b) -> p haw b", haw=S // 2, b=2)
                    t1 = ptpool.tile([128, S // 2], f32, tag="tp1")
                    nc.vector.tensor_reduce(t1[:], x5a, AX.X, ALU.add)
                    t1v = t1[:].rearrange(
                        "p (h a w) -> p h w a", h=HH // 2, a=2, w=WW // 2)
                    nc.vector.tensor_reduce(t[:, HH + WW:NKV], t1v, AX.X,
                                            ALU.add)
                    xp.append(t)

                # ---- k projection: k_pool [o, n] then sigmoid ----
                # pooling mean scales: cols 0-63 (th,tw) x 1/32, 64-319 (tp)
                # x 1/4, folded into the sigmoid's input scale
                ksig = []
                for ot in range(4):
                    kq = ps1.tile([128, NKV], f32, tag="ps1")
                    for ct in range(4):
                        nc.tensor.matmul(
                            kq[:], wk_s[ct][:, ot * 128:(ot + 1) * 128],
                            xp[ct][:], start=(ct == 0), stop=(ct == 3))
                    t = kqpool.tile([128, NKV], bf16, tag=f"ksig{ot}")
                    nc.scalar.activation(t[:, 0:2 * HH], kq[:, 0:2 * HH],
                                         AF.Sigmoid, scale=1.0 / WW)
                    nc.scalar.activation(t[:, 2 * HH:NKV], kq[:, 2 * HH:NKV],
                                         AF.Sigmoid, scale=0.25)
                    ksig.append(t)

                # ---- v projection transposed: vT [n, o], sigmoid ----
                # vaug layout per n-tile: [128, 8 heads * 128]; per head the
                # first 64 cols are v_h^T, cols 64..127 are all-ones. Used as
                # PV lhsT [n, 128]: output rows 0-63 = attn@v (transposed),
                # rows 64-127 = softmax denominator broadcast (free on PE).
                vaug = []
                for nt in range(3):
                    pn = N_TILES[nt]
                    vv = ps1.tile([128, C], f32, tag="ps1")
                    for ct in range(4):
                        nc.tensor.matmul(
                            vv[:pn, :],
                            xp[ct][:, N_OFFS[nt]:N_OFFS[nt] + pn],
                            wv_s[ct][:], start=(ct == 0), stop=(ct == 3))
                    t = vpool.tile([128, HEADS * 128], bf16, tag=f"vaug{nt}")
                    t3 = t[:pn, :].rearrange("p (h d) -> p h d", h=HEADS,
                                             d=128)
                    nc.vector.memset(t3[:, :, D:128], 1.0)
                    vx3 = vv[:pn, :].rearrange("p (h d) -> p h d", h=HEADS,
                                               d=D)
                    # per-partition scale AP on n-tile 0 (th/tw rows x 1/32,
                    # tp rows x 1/4); tiles 1,2 are all-tp (x 1/4)
                    nc.scalar.activation(t3[:, :, 0:D], vx3, AF.Sigmoid,
                                         scale=(vsc0[:] if nt == 0 else 0.25))
                    vaug.append(t)

                # ---- q projection: q [o, s] bf16 ----
                qs = []
                for ot in range(4):
                    t = xbpool.tile([128, S], bf16, tag=f"qs{ot}")
                    qq = [ps1.tile([128, 512], f32, tag="ps1",
                                   name=f"qq{e}_{ot}_{i}") for i in range(2)]
                    for ct in range(4):
                        for sh in range(2):
                            nc.tensor.matmul(
                                qq[sh][:],
                                wq_s[ct][:, ot * 128:(ot + 1) * 128],
                                xb[ct][:, sh * 512:(sh + 1) * 512],
                                start=(ct == 0), stop=(ct == 3),
                                skip_group_check=True)
                    for sh in range(2):
                        nc.vector.tensor_copy(
                            t[:, sh * 512:(sh + 1) * 512], qq[sh][:])
                    qs.append(t)

                # ---- attention per head ----
                aoC = []
                for ct in range(4):
                    t = aopool.tile([128, S], bf16, tag=f"aoC{ct}")
                    aoC.append(t)
                for h in range(8):
                    ot, ro = h // 2, (h % 2) * D
                    qh = qs[ot][ro:ro + D, :]
                    eab_h = []
                    for nt in range(3):
                        pn = N_TILES[nt]
                        dt_ = psd.tile([128, S], f32, tag="psd")
                        kh = ksig[ot][ro:ro + D,
                                      N_OFFS[nt]:N_OFFS[nt] + pn]
                        for sh in range(2):
                            nc.tensor.matmul(
                                dt_[:pn, sh * 512:(sh + 1) * 512], kh,
                                qh[:, sh * 512:(sh + 1) * 512],
                                start=True, stop=True)
                        ea = eapool.tile([128, S], bf16, tag="ea")
                        nc.scalar.activation(ea[:pn, :], dt_[:pn, :], AF.Exp)
                        eb = eabpool.tile([128, S], bf16, tag="eab")
                        eng = nc.gpsimd if nt == 1 else nc.vector
                        eng.tensor_mul(eb[:pn, :], ea[:pn, :], ep_s[nt][:])
                        eab_h.append(eb)
                    # PV: out rows 0-63 = (attn@v)^T, rows 64-127 = denom
                    pv = psv.tile([128, S], f32, tag="psv")
                    for nt in range(3):
                        pn = N_TILES[nt]
                        for sh in range(2):
                            nc.tensor.matmul(
                                pv[:, sh * 512:(sh + 1) * 512],
                                vaug[nt][:pn, h * 128:(h + 1) * 128],
                                eab_h[nt][:pn, sh * 512:(sh + 1) * 512],
                                start=(nt == 0), stop=(nt == 2),
                                skip_group_check=True)
                    dsb = eapool.tile([128, S], f32, tag="dsb")
                    nc.scalar.copy(dsb[:D, :], pv[D:2 * D, :])
                    rden = eapool.tile([128, S], f32, tag="rden")
                    nc.vector.reciprocal_approx_fast(rden[:D, :], dsb[:D, :])
                    nc.vector.tensor_mul(aoC[ot][ro:ro + D, :], pv[0:D, :],
                                         rden[:D, :])

                # ---- output projection + bias, DMA out ----
                for ot in range(4):
                    t = ofpool.tile([128, S], f32, tag="ofin")
                    oo = [ps1.tile([128, 512], f32, tag="ps1",
                                   name=f"oo{e}_{ot}_{i}") for i in range(2)]
                    for ct in range(4):
                        for sh in range(2):
                            nc.tensor.matmul(
                                oo[sh][:],
                                wo_s[ct][:, ot * 128:(ot + 1) * 128],
                                aoC[ct][:, sh * 512:(sh + 1) * 512],
                                start=(ct == 0), stop=(ct == 3),
                                skip_group_check=True)
                    for sh in range(2):
                        nc.vector.tensor_scalar_add(
                            t[:, sh * 512:(sh + 1) * 512], oo[sh][:],
                            bo_s[ot][:])
                    nc.gpsimd.dma_start(
                        out_d[e, ot * 128:(ot + 1) * 128, :], t[:])
    nc.compile()
    return nc


TRACE = False
TRACE_DIR = None


def kernel(x, Wqkv, Wout, bout, pos_embed):
    from concourse.bass_utils import run_bass_kernel_spmd

    bf = ml_dtypes.bfloat16
    scale = D ** (-0.5)
    WqT = np.ascontiguousarray((Wqkv[0:C].T * scale).astype(bf))
    WkT = np.ascontiguousarray(Wqkv[C:2 * C].T.astype(bf))
    WvT = np.ascontiguousarray(Wqkv[2 * C:3 * C].T.astype(bf))
    WoT = np.ascontiguousarray(Wout.T.astype(bf))
    boc = np.ascontiguousarray(bout.reshape(C, 1).astype(np.float32))
    eposT = np.ascontiguousarray(
        np.exp(pos_embed[0].astype(np.float64)).T.astype(bf))
    xs = np.ascontiguousarray(x.reshape(B, C, S).astype(np.float32))

    in_maps = [
        dict(x=np.ascontiguousarray(xs[i * BPC:(i + 1) * BPC]), wqT=WqT,
             wkT=WkT, wvT=WvT, woutT=WoT, bout=boc, eposT=eposT)
        for i in range(NCORES)
    ]
    nc = _build_graph()
    kw = {}
    if TRACE:
        kw = dict(trace=True, tmpdir=TRACE_DIR)
    res = run_bass_kernel_spmd(nc, in_maps, core_ids=list(range(NCORES)), **kw)
    if TRACE:
        print(f"HW exec time: {res.exec_time_ns} ns")
    outs = [np.asarray(r["out"], dtype=np.float32) for r in res.results]
    return np.concatenate(outs, axis=0).reshape(B, C, HH, WW)


# revision 31
# speedup vs baseline: 1.1736x; 1.1736x over previous
"""Trainium2 Bass kernel for pooled-kv sparse attention (nn_Attention).

Math (per batch element, C=512, HEADS=8, d=64, H=W=32, s=1024, n=320):
  qkv = Wqkv @ x  (1x1 conv) ; k,v pooled over space to n=320 then sigmoid
  dots = (q^T k)/8 + pos ; attn = softmax(dots) ; out = Wout @ (attn v) + bout

Device strategy (data parallel, 4 batch elements per core, 8 cores):
  - Pooling commutes with the 1x1 conv: pool x first (DVE), then project.
  - All matmuls in bf16 (PSUM accumulates f32).
  - Attention computed with n on partitions for dots (dotsT [n, s]), exp on
    ACT, multiply by host-precomputed exp(posT) on GpSimd
    (exp(dots+pos) == exp(dots)*exp(pos)), PV with s on partitions via
    lhsT=exp_attn, rhs=[v^T | 1] (ones column yields softmax denominators).
  - Normalization folded into the PSUM->SBUF copy (tensor_scalar by 1/den).
  - ao (s-partitioned) transposed to c-partitioned via DMA transpose (bf16).
"""

import os
import sys

import numpy as np
import ml_dtypes

sys.path.insert(0, "/opt/trn_rl_repo")

HEADS = 8
C = 512
HH = 32
WW = 32
S = HH * WW          # 1024
NKV = HH + WW + (HH // 2) * (WW // 2)  # 320
D = C // HEADS       # 64
B = 32
NCORES = 8
BPC = B // NCORES    # 4 batch elements per core

N_TILES = (128, 128, 64)       # n=320 split over partition tiles
N_OFFS = (0, 128, 256)


def _build_graph():
    import concourse.bass as bass
    import concourse.tile as tile
    from concourse import bacc, mybir

    f32 = mybir.dt.float32
    bf16 = mybir.dt.bfloat16
    AF = mybir.ActivationFunctionType
    ALU = mybir.AluOpType
    AX = mybir.AxisListType

    nc = bacc.Bacc("TRN2", target_bir_lowering=False, debug=False)

    x_d = nc.dram_tensor("x", [BPC, C, S], f32, kind="ExternalInput").ap()
    wq_d = nc.dram_tensor("wqT", [C, C], bf16, kind="ExternalInput").ap()
    wk_d = nc.dram_tensor("wkT", [C, C], bf16, kind="ExternalInput").ap()
    wv_d = nc.dram_tensor("wvT", [C, C], bf16, kind="ExternalInput").ap()
    wo_d = nc.dram_tensor("woutT", [C, C], bf16, kind="ExternalInput").ap()
    bo_d = nc.dram_tensor("bout", [C, 1], f32, kind="ExternalInput").ap()
    ep_d = nc.dram_tensor("eposT", [NKV, S], bf16, kind="ExternalInput").ap()
    out_d = nc.dram_tensor("out", [BPC, C, S], f32, kind="ExternalOutput").ap()

    with tile.TileContext(nc) as tc, nc.allow_low_precision("bf16 by design"):
        with (
            tc.tile_pool(name="const", bufs=1) as cpool,
            tc.tile_pool(name="xf", bufs=3) as xfpool,
            tc.tile_pool(name="xb", bufs=2) as xbpool,
            tc.tile_pool(name="xp", bufs=2) as xppool,
            tc.tile_pool(name="ptmp", bufs=3) as ptpool,
            tc.tile_pool(name="kq", bufs=2) as kqpool,
            tc.tile_pool(name="ea", bufs=3) as eapool,
            tc.tile_pool(name="eab", bufs=3) as eabpool,
            tc.tile_pool(name="ao", bufs=2) as aopool,
            tc.tile_pool(name="ofin", bufs=2) as ofpool,
            tc.tile_pool(name="vaug", bufs=2) as vpool,
            tc.tile_pool(name="psd", bufs=2, space="PSUM") as psd,
            tc.tile_pool(name="psv", bufs=2, space="PSUM") as psv,
        ):
            # ---- constants (loaded once) ----
            wq_s, wk_s, wv_s, wo_s, bo_s = [], [], [], [], []
            for ct in range(4):
                for wi, (lst, dram) in enumerate(
                        ((wq_s, wq_d), (wk_s, wk_d), (wv_s, wv_d),
                         (wo_s, wo_d))):
                    t = cpool.tile([128, C], bf16, tag=f"w{wi}_{ct}")
                    nc.sync.dma_start(t[:], dram[ct * 128:(ct + 1) * 128, :])
                    lst.append(t)
                t = cpool.tile([128, 1], f32, tag=f"bo_{ct}")
                nc.sync.dma_start(t[:], bo_d[ct * 128:(ct + 1) * 128, :])
                bo_s.append(t)
            ep_s = []
            for nt in range(3):
                t = cpool.tile([N_TILES[nt], S], bf16, tag=f"ep_{nt}")
                nc.sync.dma_start(
                    t[:], ep_d[N_OFFS[nt]:N_OFFS[nt] + N_TILES[nt], :])
                ep_s.append(t)
            # per-partition sigmoid scale for v-proj n-tile 0
            vsc0 = cpool.tile([128, 1], f32, tag="vsc0")
            nc.vector.memset(vsc0[0:2 * HH, :], 1.0 / WW)
            nc.vector.memset(vsc0[2 * HH:128, :], 0.25)

            for e in range(BPC):
                # ---- load x, cast to bf16 (GpSimd) ----
                xb = []
                for ct in range(4):
                    xf = xfpool.tile([128, S], f32, tag="xf")
                    nc.gpsimd.dma_start(
                        xf[:], x_d[e, ct * 128:(ct + 1) * 128, :])
                    t = xbpool.tile([128, S], bf16, tag=f"xb{ct}")
                    nc.gpsimd.tensor_copy(t[:], xf[:])
                    xb.append(t)

                # ---- pooling -> xp [c, 320] bf16 raw SUMS (DVE) ----
                # (the 1/32, 1/32, 0.25 mean scales are folded into the
                # sigmoid activations downstream)
                xp = []
                for ct in range(4):
                    t = xppool.tile([128, NKV], bf16, tag=f"xp{ct}")
                    # tp1[h, w2] = x[h, 2w2] + x[h, 2w2+1]
                    x5a = xb[ct][:].rearrange(
                        "p (haw b) -> p haw b", haw=S // 2, b=2)
                    t1 = ptpool.tile([128, S // 2], f32, tag="tp1")
                    nc.vector.tensor_reduce(t1[:], x5a, AX.X, ALU.add)
                    # th[h] = sum_w2 tp1[h, w2]
                    t1h = t1[:].rearrange("p (h w2) -> p h w2", h=HH,
                                          w2=WW // 2)
                    nc.vector.tensor_reduce(t[:, 0:HH], t1h, AX.X, ALU.add)
                    # tw: sum over h of x (strided view, h innermost)
                    xwh = xb[ct][:].rearrange("p (h w) -> p w h", h=HH, w=WW)
                    nc.vector.tensor_reduce(t[:, HH:HH + WW], xwh, AX.X,
                                            ALU.add)
                    # tp[h2, w2] = tp1[2h2, w2] + tp1[2h2+1, w2]
                    t1v = t1[:].rearrange(
                        "p (h a w) -> p h w a", h=HH // 2, a=2, w=WW // 2)
                    nc.vector.tensor_reduce(t[:, HH + WW:NKV], t1v, AX.X,
                                            ALU.add)
                    xp.append(t)

                # ---- k projection: k_pool [o, n] then sigmoid ----
                # pooling mean scales: cols 0-63 (th,tw) x 1/32, 64-319 (tp)
                # x 1/4, folded into the sigmoid's input scale
                ksig = []
                for ot in range(4):
                    kq = psd.tile([128, S], f32, tag="psd", name=f"kq{e}_{ot}")
                    for ct in range(4):
                        nc.tensor.matmul(
                            kq[:, 0:NKV],
                            wk_s[ct][:, ot * 128:(ot + 1) * 128],
                            xp[ct][:], start=(ct == 0), stop=(ct == 3))
                    t = kqpool.tile([128, NKV], bf16, tag=f"ksig{ot}")
                    nc.scalar.activation(t[:, 0:2 * HH], kq[:, 0:2 * HH],
                                         AF.Sigmoid, scale=1.0 / WW)
                    nc.scalar.activation(t[:, 2 * HH:NKV], kq[:, 2 * HH:NKV],
                                         AF.Sigmoid, scale=0.25)
                    del kq
                    ksig.append(t)

                # ---- v projection transposed: vT [n, o], sigmoid ----
                # vaug layout per n-tile: [128, 8 heads * 128]; per head the
                # first 64 cols are v_h^T, cols 64..127 are all-ones. Used as
                # PV lhsT [n, 128]: output rows 0-63 = attn@v (transposed),
                # rows 64-127 = softmax denominator broadcast (free on PE).
                vaug = []
                for nt in range(3):
                    pn = N_TILES[nt]
                    vv = psd.tile([128, S], f32, tag="psd", name=f"vv{e}_{nt}")
                    for ct in range(4):
                        nc.tensor.matmul(
                            vv[:pn, 0:C],
                            xp[ct][:, N_OFFS[nt]:N_OFFS[nt] + pn],
                            wv_s[ct][:], start=(ct == 0), stop=(ct == 3))
                    t = vpool.tile([128, HEADS * 128], bf16, tag=f"vaug{nt}")
                    t3 = t[:pn, :].rearrange("p (h d) -> p h d", h=HEADS,
                                             d=128)
                    nc.vector.memset(t3[:, :, D:128], 1.0)
                    vx3 = vv[:pn, 0:C].rearrange("p (h d) -> p h d", h=HEADS,
                                                 d=D)
                    # per-partition scale AP on n-tile 0 (th/tw rows x 1/32,
                    # tp rows x 1/4); tiles 1,2 are all-tp (x 1/4)
                    nc.scalar.activation(t3[:, :, 0:D], vx3, AF.Sigmoid,
                                         scale=(vsc0[:] if nt == 0 else 0.25))
                    vaug.append(t)

                # ---- q projection: q [o, s] bf16 ----
                qs = []
                for ot in range(4):
                    t = xbpool.tile([128, S], bf16, tag=f"qs{ot}")
                    qq0 = psd.tile([128, S], f32, tag="psd", name=f"qq{e}_{ot}")
                    qq = [qq0[:, 0:512], qq0[:, 512:1024]]
                    for ct in range(4):
                        for sh in range(2):
                            nc.tensor.matmul(
                                qq[sh],
                                wq_s[ct][:, ot * 128:(ot + 1) * 128],
                                xb[ct][:, sh * 512:(sh + 1) * 512],
                                start=(ct == 0), stop=(ct == 3),
                                skip_group_check=True)
                    nc.vector.tensor_copy(t[:], qq0[:])
                    qs.append(t)

                # ---- attention per head ----
                aoC = []
                for ct in range(4):
                    t = aopool.tile([128, S], bf16, tag=f"aoC{ct}")
                    aoC.append(t)
                for h in range(8):
                    ot, ro = h // 2, (h % 2) * D
                    qh = qs[ot][ro:ro + D, :]
                    eab_h = []
                    for nt in range(3):
                        pn = N_TILES[nt]
                        dt_ = psd.tile([128, S], f32, tag="psd")
                        kh = ksig[ot][ro:ro + D,
                                      N_OFFS[nt]:N_OFFS[nt] + pn]
                        for sh in range(2):
                            nc.tensor.matmul(
                                dt_[:pn, sh * 512:(sh + 1) * 512], kh,
                                qh[:, sh * 512:(sh + 1) * 512],
                                start=True, stop=True)
                        ea = eapool.tile([128, S], bf16, tag="ea")
                        nc.scalar.activation(ea[:pn, :], dt_[:pn, :], AF.Exp)
                        eb = eabpool.tile([128, S], bf16, tag="eab")
                        eng = nc.gpsimd if nt == 1 else nc.vector
                        eng.tensor_mul(eb[:pn, :], ea[:pn, :], ep_s[nt][:])
                        eab_h.append(eb)
                    # PV: out rows 0-63 = (attn@v)^T, rows 64-127 = denom
                    pv = psv.tile([128, S], f32, tag="psv")
                    for nt in range(3):
                        pn = N_TILES[nt]
                        for sh in range(2):
                            nc.tensor.matmul(
                                pv[:, sh * 512:(sh + 1) * 512],
                                vaug[nt][:pn, h * 128:(h + 1) * 128],
                                eab_h[nt][:pn, sh * 512:(sh + 1) * 512],
                                start=(nt == 0), stop=(nt == 2),
                                skip_group_check=True)
                    dsb = eapool.tile([128, S], f32, tag="dsb")
                    nc.scalar.copy(dsb[:D, :], pv[D:2 * D, :])
                    rden = eapool.tile([128, S], f32, tag="rden")
                    nc.vector.reciprocal_approx_fast(rden[:D, :], dsb[:D, :])
                    nc.vector.tensor_mul(aoC[ot][ro:ro + D, :], pv[0:D, :],
                                         rden[:D, :])

                # ---- output projection + bias, DMA out ----
                for ot in range(4):
                    t = ofpool.tile([128, S], f32, tag="ofin")
                    oo0 = psd.tile([128, S], f32, tag="psd", name=f"oo{e}_{ot}")
                    oo = [oo0[:, 0:512], oo0[:, 512:1024]]
                    for ct in range(4):
                        for sh in range(2):
                            nc.tensor.matmul(
                                oo[sh],
                                wo_s[ct][:, ot * 128:(ot + 1) * 128],
                                aoC[ct][:, sh * 512:(sh + 1) * 512],
                                start=(ct == 0), stop=(ct == 3),
                                skip_group_check=True)
                    nc.vector.tensor_scalar_add(t[:], oo0[:], bo_s[ot][:])
                    nc.gpsimd.dma_start(
                        out_d[e, ot * 128:(ot + 1) * 128, :], t[:])
    nc.compile()
    return nc


TRACE = False
TRACE_DIR = None


def kernel(x, Wqkv, Wout, bout, pos_embed):
    from concourse.bass_utils import run_bass_kernel_spmd

    bf = ml_dtypes.bfloat16
    scale = D ** (-0.5)
    WqT = np.ascontiguousarray((Wqkv[0:C].T * scale).astype(bf))
    WkT = np.ascontiguousarray(Wqkv[C:2 * C].T.astype(bf))
    WvT = np.ascontiguousarray(Wqkv[2 * C:3 * C].T.astype(bf))
    WoT = np.ascontiguousarray(Wout.T.astype(bf))
    boc = np.ascontiguousarray(bout.reshape(C, 1).astype(np.float32))
    eposT = np.ascontiguousarray(
        np.exp(pos_embed[0].astype(np.float64)).T.astype(bf))
    xs = np.ascontiguousarray(x.reshape(B, C, S).astype(np.float32))

    in_maps = [
        dict(x=np.ascontiguousarray(xs[i * BPC:(i + 1) * BPC]), wqT=WqT,
             wkT=WkT, wvT=WvT, woutT=WoT, bout=boc, eposT=eposT)
        for i in range(NCORES)
    ]
    nc = _build_graph()
    kw = {}
    if TRACE:
        kw = dict(trace=True, tmpdir=TRACE_DIR)
    res = run_bass_kernel_spmd(nc, in_maps, core_ids=list(range(NCORES)), **kw)
    if TRACE:
        print(f"HW exec time: {res.exec_time_ns} ns")
    outs = [np.asarray(r["out"], dtype=np.float32) for r in res.results]
    return np.concatenate(outs, axis=0).reshape(B, C, HH, WW)
